# revision 19
# baseline (speedup 1.0000x reference)
"""Trainium2 Bass kernel for nn_Attention_31267361915369.

Computation (per batch example, T=4096, D=1024):
    h   = tanh(x @ W1.T + b1)          # [T, D]
    s   = h @ w2.T + b2                # [T]
    e   = exp(s)                       # no max-subtraction: |s| <= sum|w2| ~ 26,
                                       # and num/den is exactly shift-invariant
    num = cumsum(e * x, axis=0)        # [T, D]
    den = cumsum(e)                    # [T]
    ctx = num / den
    out = tanh([ctx, x] @ Wc.T + bc)   # [T, D]

Key reformulation: split Wc = [Wc1 | Wc2] (ctx half, x half). Right-matmul
commutes with the t-cumsum, so
    ctx @ Wc1.T = cumsum(e * (x @ Wc1.T)) / den = cumsum(e * y) / den
and
    out = tanh(cumsum(e*y)/den + x @ Wc2.T + bc).
Versus the direct form this removes the 8 PE transposes of ctx and the
ctxT staging copy per tile (the cumsum result feeds the output through
pure element-wise ops), and the natural-layout x input disappears (x is
only ever consumed through the host-pretransposed xT).

Distribution: data-parallel over batch B=8 across the 8 NeuronCores (one
example per core), weights replicated. No collectives.

Per-core dataflow (32 token-tiles of 128):
  - all matmuls run in bf16 (fp32 PSUM accumulation); measured end-to-end
    rel err ~2e-3.
  - stage_a(i): mm1 (h_pre) + y GEMM + scores + ex=[e*y | e];
    stage_b(i): causal prefix sums (upper-triangular ones matmul), carry
    chain, xwc2 GEMM, tail tanh(z*rden + xwc2). Stage-skewed emission:
    tile i's stage_b overlaps tile i+1's stage_a on TensorE.
  - the running carry is injected into ex row 0 (tri row 0 is all ones,
    so the triangular matmul propagates it to every output row); the
    carry row is extracted from PSUM rows [96:128] via ACT copy + a
    SBUF->SBUF DMA of row 127.
"""

import sys

if "/opt/trn_rl_repo" not in sys.path:
    sys.path.insert(0, "/opt/trn_rl_repo")

from contextlib import ExitStack

import ml_dtypes
import numpy as np

import concourse.bass as bass
import concourse.tile as tile
from concourse import bacc, mybir
from concourse.bass_utils import run_bass_kernel_spmd

P = 128
D = 1024
T_FULL = 4096
N_CORES = 8

BF = mybir.dt.bfloat16
F32 = mybir.dt.float32
AFT = mybir.ActivationFunctionType
ALU = mybir.AluOpType

_BUILD_CACHE: dict = {}


def build(T: int = T_FULL, use_b1: bool = False, use_bc: bool = False,
          repeat: int = 1, fp8_h: bool = False):
    """Build + compile the per-core Bass program for a [T, D] shard."""
    key = (T, use_b1, use_bc, repeat)
    if key in _BUILD_CACHE:
        return _BUILD_CACHE[key]

    assert T % P == 0
    NT = T // P
    NK = D // P           # 8 k-tiles per D-wide contraction

    nc = bacc.Bacc("TRN2", target_bir_lowering=False, debug=False)

    # host-pretransposed x: xt[i, p, k, t] = x[i*128+t, k*128+p]
    xt_d = nc.declare_dram_parameter("xt", [T * NK, P], BF, isOutput=False)
    w1t_d = nc.declare_dram_parameter("w1t", [D, D], BF, isOutput=False)
    wct_d = nc.declare_dram_parameter("wct", [2 * D, D], BF, isOutput=False)
    w2r_d = nc.declare_dram_parameter("w2r", [P, D], F32, isOutput=False)
    tri_d = nc.declare_dram_parameter("tri", [P, P], BF, isOutput=False)
    b1_d = nc.declare_dram_parameter("b1r", [1, D], BF, isOutput=False) if use_b1 else None
    bc_d = nc.declare_dram_parameter("bcr", [1, D], BF, isOutput=False) if use_bc else None
    out_d = nc.declare_dram_parameter("out", [T, D], F32, isOutput=True)

    xt_t = xt_d.ap().rearrange("(n p k) q -> n p k q", p=P, k=NK)
    out_t = out_d.ap().rearrange("(n p) d -> n p d", p=P)
    w1_t = w1t_d.ap().rearrange("(k p) e -> k p e", p=P)
    wc_t = wct_d.ap().rearrange("(k p) e -> k p e", p=P)

    with tile.TileContext(nc) as tc, ExitStack() as ctx:
        consts = ctx.enter_context(tc.tile_pool(name="consts", bufs=1))
        xtp = ctx.enter_context(tc.tile_pool(name="xtp", bufs=4))
        hpool = ctx.enter_context(tc.tile_pool(name="hpool", bufs=2))
        scr = ctx.enter_context(tc.tile_pool(name="scr", bufs=2))
        expool = ctx.enter_context(tc.tile_pool(name="expool", bufs=2))
        upool = ctx.enter_context(tc.tile_pool(name="upool", bufs=2))
        outp = ctx.enter_context(tc.tile_pool(name="outp", bufs=2))
        colp = ctx.enter_context(tc.tile_pool(name="colp", bufs=4))
        carryp = ctx.enter_context(tc.tile_pool(name="carryp", bufs=2))
        crowp = ctx.enter_context(tc.tile_pool(name="crowp", bufs=2))
        ph = ctx.enter_context(tc.tile_pool(name="ph", bufs=1, space="PSUM"))
        py = ctx.enter_context(tc.tile_pool(name="py", bufs=1, space="PSUM"))
        pc = ctx.enter_context(tc.tile_pool(name="pc", bufs=1, space="PSUM"))
        po = ctx.enter_context(tc.tile_pool(name="po", bufs=1, space="PSUM"))

        # constants / weights (small ones first: needed earliest)
        tri_sb = consts.tile([P, P], BF, tag="tri")
        nc.sync.dma_start(out=tri_sb[:], in_=tri_d.ap())
        # f32: wide bf16 TensorTensor/TensorReduce DVE ops hang on this hw
        w2r_sb = consts.tile([P, D], F32, tag="w2r")
        nc.sync.dma_start(out=w2r_sb[:], in_=w2r_d.ap())
        if use_b1:
            b1_sb = consts.tile([1, D], BF, tag="b1")
            nc.sync.dma_start(out=b1_sb[:], in_=b1_d.ap())
        if use_bc:
            bc_sb = consts.tile([1, D], BF, tag="bc")
            nc.sync.dma_start(out=bc_sb[:], in_=bc_d.ap())
        w1_sb = []
        for k in range(NK):
            t = consts.tile([P, D], BF, tag=f"w1_{k}")
            nc.sync.dma_start(out=t[:], in_=w1_t[k])
            w1_sb.append(t)
        wc_sb = []
        for k in range(2 * NK):
            t = consts.tile([P, D], BF, tag=f"wc_{k}")
            nc.sync.dma_start(out=t[:], in_=wc_t[k])
            wc_sb.append(t)

        carry_tiles = {}

        def stage_a(i):
            """load + scores + ex=[e*y | e] for tile i -> (xT, ex)"""
            xT = xtp.tile([P, NK, P], BF, tag="xt")
            nc.sync.dma_start(out=xT[:], in_=xt_t[i])

            ph_t = ph.tile([P, D], F32, tag="ph")
            for k in range(NK):
                last = k == NK - 1 and not use_b1
                for c in range(2):
                    nc.tensor.matmul(
                        ph_t[:, c * 512:(c + 1) * 512],
                        xT[:, k, :],
                        w1_sb[k][:, c * 512:(c + 1) * 512],
                        start=(k == 0),
                        stop=last,
                    )
            if use_b1:
                for c in range(2):
                    nc.tensor.matmul(
                        ph_t[:, c * 512:(c + 1) * 512],
                        tri_sb[0:1, :],
                        b1_sb[0:1, c * 512:(c + 1) * 512],
                        start=False,
                        stop=True,
                    )
            # y = x @ Wc1.T (ctx half of Wc, pre-cumsum)
            py_t = py.tile([P, D], F32, tag="py")
            for k in range(NK):
                for c in range(2):
                    nc.tensor.matmul(
                        py_t[:, c * 512:(c + 1) * 512],
                        xT[:, k, :],
                        wc_sb[k][:, c * 512:(c + 1) * 512],
                        start=(k == 0),
                        stop=(k == NK - 1),
                    )

            h_sb = hpool.tile([P, D], F32, tag="h")
            nc.scalar.activation(h_sb[:], ph_t[:], AFT.Tanh)

            s_col = colp.tile([P, 1], F32, tag="s")
            prod = scr.tile([P, D], F32, tag="scr")
            nc.vector.tensor_mul(prod[:], h_sb[:], w2r_sb[:])
            nc.vector.reduce_sum(s_col[:], prod[:], axis=mybir.AxisListType.X)
            e_col = colp.tile([P, 1], F32, tag="e")
            nc.scalar.activation(e_col[:], s_col[:], AFT.Exp)
            ex_sb = expool.tile([P, D + 1], BF, tag="ex")
            nc.scalar.copy(ex_sb[:, D:D + 1], e_col[:])
            nc.vector.tensor_scalar_mul(ex_sb[:, 0:D], py_t[:], e_col[:])
            return xT, ex_sb

        def stage_b1(i, xT, ex_sb):
            """cumsum + carry chain for tile i -> (xT, pc_t, rden)"""
            if i > 0:
                # inject the running carry into ex row 0: tri[0, t] = 1 for
                # all t, so the triangular matmul propagates it to every
                # output row.
                nc.vector.tensor_add(
                    ex_sb[0:1, :], ex_sb[0:1, :], carry_tiles[i - 1][0:1, :]
                )
            pc_t = pc.tile([P, D], F32, tag="pc")
            # same tag as po_t: they share the single po-pool buffer. pd is
            # fully consumed (extract + reciprocal) in this stage; po_t's
            # GEMM sits a whole stage_a later in the PE stream, so the
            # pool's serialization on pd's readers costs no PE stall.
            pd_t = po.tile([P, 1], F32, tag="po")
            for c in range(2):
                nc.tensor.matmul(
                    pc_t[:, c * 512:(c + 1) * 512],
                    tri_sb[:],
                    ex_sb[:, c * 512:(c + 1) * 512],
                    start=True,
                    stop=True,
                )
            nc.tensor.matmul(
                pd_t[:], tri_sb[:], ex_sb[:, D:D + 1], start=True, stop=True
            )

            # extract running totals (row 127 of PSUM) for the next tile's
            # carry. engines can't move data across partitions (and must start
            # at a 32-aligned partition), so copy the [96:128] window.
            if i < NT - 1:
                cstage = carryp.tile([P, D + 1], BF, tag="carry")
                nc.scalar.copy(cstage[96:128, 0:D], pc_t[96:128, :])
                nc.scalar.copy(cstage[96:128, D:D + 1], pd_t[96:128, :])
                crow = crowp.tile([1, D + 1], BF, tag="crow")
                nc.sync.dma_start(out=crow[0:1, :], in_=cstage[127:128, :])
                carry_tiles[i] = crow

            rden = colp.tile([P, 1], F32, tag="rden")
            nc.vector.reciprocal(rden[:], pd_t[:])
            return xT, pc_t, rden

        def stage_b2(i, xT, pc_t, rden):
            """xwc2 GEMM + tail for tile i"""
            # xwc2 = x @ Wc2.T
            po_t = po.tile([P, D], F32, tag="po")
            for k in range(NK):
                last = k == NK - 1 and not use_bc
                for c in range(2):
                    nc.tensor.matmul(
                        po_t[:, c * 512:(c + 1) * 512],
                        xT[:, k, :],
                        wc_sb[NK + k][:, c * 512:(c + 1) * 512],
                        start=(k == 0),
                        stop=last,
                    )
            if use_bc:
                for c in range(2):
                    nc.tensor.matmul(
                        po_t[:, c * 512:(c + 1) * 512],
                        tri_sb[0:1, :],
                        bc_sb[0:1, c * 512:(c + 1) * 512],
                        start=False,
                        stop=True,
                    )

            # tail: out = tanh(z*rden + xwc2)
            u_sb = upool.tile([P, D], F32, tag="u")
            nc.vector.tensor_scalar_mul(u_sb[:], pc_t[:], rden[:])
            nc.vector.tensor_add(u_sb[:], u_sb[:], po_t[:])
            o_sb = outp.tile([P, D], F32, tag="out")
            nc.scalar.activation(o_sb[:], u_sb[:], AFT.Tanh)
            nc.sync.dma_start(out=out_t[i], in_=o_sb[:])

        def whole_pipeline():
            carry_tiles.clear()
            pend_a = {}
            pend_b = {}
            for i in range(NT):
                pend_a[i] = stage_a(i)
                if i >= 2:
                    stage_b2(i - 2, *pend_b.pop(i - 2))
                if i >= 1:
                    pend_b[i - 1] = stage_b1(i - 1, *pend_a.pop(i - 1))
            stage_b2(NT - 2, *pend_b.pop(NT - 2))
            pend_b[NT - 1] = stage_b1(NT - 1, *pend_a.pop(NT - 1))
            stage_b2(NT - 1, *pend_b.pop(NT - 1))

        if repeat == 1:
            whole_pipeline()
        else:
            with tc.For_i(0, repeat, 1):
                whole_pipeline()

    nc.compile()
    _BUILD_CACHE[key] = nc
    return nc


def _bf16(a):
    return np.ascontiguousarray(np.asarray(a, dtype=np.float32)).astype(
        ml_dtypes.bfloat16
    )


def make_in_maps(x, W1, b1, w2, b2, Wc, bc, T=T_FULL):
    """Host-side prep: shard x over batch, pre-transpose/replicate weights."""
    x = np.asarray(x, dtype=np.float32)
    W1 = np.asarray(W1, dtype=np.float32)
    Wc = np.asarray(Wc, dtype=np.float32)
    w2 = np.asarray(w2, dtype=np.float32).reshape(1, -1)
    b1 = np.asarray(b1, dtype=np.float32)
    bc = np.asarray(bc, dtype=np.float32)
    use_b1 = bool(np.any(b1 != 0.0))
    use_bc = bool(np.any(bc != 0.0))
    # b2 shifts every score equally; exp(b2) cancels in num/den.

    w1t = _bf16(W1.T)
    # wct rows: k<1024 -> Wc1.T (ctx half, consumed pre-cumsum as y),
    #           k>=1024 -> Wc2.T (x half)
    wct = _bf16(Wc.T)
    w2r = np.ascontiguousarray(np.broadcast_to(w2, (P, D)).astype(np.float32))
    tri = _bf16(np.triu(np.ones((P, P), np.float32)))

    NT = T // P
    NK = D // P
    in_maps = []
    for i in range(N_CORES):
        xb = _bf16(x[i, :T, :])
        # xt[i, p, k, t] = x[i*128+t, k*128+p], 2KB-contiguous per partition
        xt = np.ascontiguousarray(
            xb.reshape(NT, P, NK, P).transpose(0, 3, 2, 1)
        ).reshape(T * NK, P)
        m = {"xt": xt, "w1t": w1t, "wct": wct, "w2r": w2r, "tri": tri}
        if use_b1:
            m["b1r"] = _bf16(b1.reshape(1, D))
        if use_bc:
            m["bcr"] = _bf16(bc.reshape(1, D))
        in_maps.append(m)
    return in_maps, use_b1, use_bc


def kernel(x, W1, b1, w2, b2, Wc, bc):
    in_maps, use_b1, use_bc = make_in_maps(x, W1, b1, w2, b2, Wc, bc)
    nc = build(T_FULL, use_b1, use_bc)
    res = run_bass_kernel_spmd(nc, in_maps, core_ids=list(range(N_CORES)))
    out = np.stack([np.asarray(res.results[i]["out"]) for i in range(N_CORES)], axis=0)
    return out.astype(np.float32)
